# revision 46
# baseline (speedup 1.0000x reference)
"""AttentionWithBinding distributed Bass kernel for 8 TRN2 NeuronCores.

Sharding: 8 cores = 2 batches x 4 head-groups (4 heads / 256 dims each).
Per core: q/k/v projections (weight-stationary matmuls from a host
pre-transposed xT), flash-style attention in scoresT [sk, sq] orientation,
softmax exp on ScalarE with the additive binding bias folded in as a
host-precomputed exp(0.5*binding.T) bf16 multiplier on VectorE, row-sums
fused into the attn@v matmul via a ones-column on v, and the per-head
o-projection partials. Host sums the 4 partials per batch and adds the
analytic bias vector bv@Wo + bo.

Structure (vs the naive per-chunk loop):
- One flat software pipeline over all (chunk, head-pair, sk-group)
  slots: scores(i), exp/mul(i-1), attn@v(i-3) with no drain/refill at
  head-pair or chunk boundaries, so neither TensorE nor ScalarE (exp is
  ~36us/chunk) ever starves.
- Input DMAs split between the sync HWDGE ring (consolidated multi-MB
  transfers; ~4us fixed cost per DMA) and the gpsimd SWDGE ring
  (fine-grained 128KB tiles at ~0.7us each), ordered by first use.
  Output staged in SBUF, written as one 1MB DMA per chunk.
- The o-projection of chunk n is deferred and dripped one (row,col)
  piece per pipeline slot through chunk n+1, providing exactly the PE
  filler slack the exp/psum alternation needs (kills HAM re-throttle
  micro-gaps); its evacuations alternate ScalarE/VectorE.
- 72 warmup matmuls bridge the initial DMA window to keep the PE clock
  gate warm (the first ~5 run cold, then HAM unthrottles).
"""

import sys

sys.path.insert(0, "/opt/trn_rl_repo")

import numpy as np
import ml_dtypes
from contextlib import ExitStack

BF16 = ml_dtypes.bfloat16

B, S, D = 2, 2048, 1024
H, HD = 16, 64
HPC = 4  # heads per core
DHC = HPC * HD  # 256 head dims per core
SCALE = HD ** -0.5
NCORES = 8
KT = D // 128  # 8 contraction tiles over D
ST = S // 128  # 16 tiles over S
CH = 512  # free-dim chunk (one PSUM bank of f32)
NQ = S // CH  # 4 query chunks

_graph_cache = {}


def _build(has_qk_bias):
    import concourse.bacc as bacc
    import concourse.mybir as mybir
    from concourse import tile

    f32 = mybir.dt.float32
    bf16 = mybir.dt.bfloat16
    AF = mybir.ActivationFunctionType

    nc = bacc.Bacc(None)

    xT_e = nc.declare_dram_parameter("xT", [D, S], bf16, isOutput=False)
    wq_e = nc.declare_dram_parameter("wq", [D, DHC], bf16, isOutput=False)
    wk_e = nc.declare_dram_parameter("wk", [D, DHC], bf16, isOutput=False)
    wv_e = nc.declare_dram_parameter("wv", [D, DHC], bf16, isOutput=False)
    wo_e = nc.declare_dram_parameter("wo", [DHC, D], bf16, isOutput=False)
    eb_e = nc.declare_dram_parameter("expbT", [S, S], bf16, isOutput=False)
    if has_qk_bias:
        bq_e = nc.declare_dram_parameter("bq", [DHC, 1], f32, isOutput=False)
        bk_e = nc.declare_dram_parameter("bk", [DHC, 1], f32, isOutput=False)
    out_e = nc.declare_dram_parameter("out", [S, D], bf16, isOutput=True)

    with tile.TileContext(nc) as tc, ExitStack() as ctx:
        const = ctx.enter_context(tc.tile_pool(name="const", bufs=1))
        xT = const.tile([128, KT, S], bf16)
        eb = const.tile([128, NQ, ST, CH], bf16)  # exp(0.5*binding).T
        wq = const.tile([128, KT, DHC], bf16)
        wk = const.tile([128, KT, DHC], bf16)
        wv = const.tile([128, KT, DHC], bf16)
        junk = const.tile([128, CH], bf16)
        wo = const.tile([128, 2, D], bf16)
        if has_qk_bias:
            bq = const.tile([128, 2], f32)
            bk = const.tile([128, 2], f32)
        qT = const.tile([128, 2, S], bf16)  # [dh, s] head-major
        kT = const.tile([128, 2, S], bf16)
        va = const.tile([128, ST, HPC, 65], bf16)  # v tiles + ones col
        outS = const.tile([128, 2, 4, D], bf16)  # output staging, 2 halves

        nc.vector.memset(junk[:], 0.0)
        # ones columns for the rowsum trick: on the (otherwise idle at
        # start) vector queue so the gpsimd ring is free for DMAs
        for s in range(ST):
            for h in range(HPC):
                nc.vector.memset(va[:, s, h, 64:65], 1.0)

        # DMA plan: sync ring carries the k-projection critical path +
        # late-need tensors; gpsimd ring carries the second xT half and
        # the chunk-0 binding tiles in parallel.
        if has_qk_bias:
            for m in range(2):
                nc.sync.dma_start(bq[:, m:m + 1], bq_e[m * 128:(m + 1) * 128, :])
                nc.sync.dma_start(bk[:, m:m + 1], bk_e[m * 128:(m + 1) * 128, :])
        nc.sync.dma_start(
            wk[:], wk_e[:, :].rearrange("(k p) c -> p k c", p=128))
        nc.sync.dma_start(
            xT[:, 0:4, :],
            xT_e[0:512, :].rearrange("(k p) c -> p k c", p=128))
        nc.gpsimd.dma_start(
            xT[:, 4:8, :],
            xT_e[512:1024, :].rearrange("(k p) c -> p k c", p=128))
        nc.sync.dma_start(
            wq[:], wq_e[:, :].rearrange("(k p) c -> p k c", p=128))
        nc.sync.dma_start(
            wv[:], wv_e[:, :].rearrange("(k p) c -> p k c", p=128))
        for t in range(ST):
            nc.gpsimd.dma_start(
                eb[:, 0, t, :], eb_e[t * 128:(t + 1) * 128, 0:CH])
        nc.sync.dma_start(
            wo[:], wo_e[:, :].rearrange("(m p) c -> p m c", p=128))
        for t in range(ST):
            nc.gpsimd.dma_start(
                eb[:, 1, t, :], eb_e[t * 128:(t + 1) * 128, CH:2 * CH])
        for n in range(2, NQ):
            nc.sync.dma_start(
                eb[:, n, :, :],
                eb_e[:, n * CH:(n + 1) * CH].rearrange(
                    "(t p) c -> p t c", p=128))

        T2 = 2  # sk tiles merged per exp/mul instruction
        GS = list(range(0, ST, T2)) + [ST]
        NG = len(GS) - 1
        psS = ctx.enter_context(tc.tile_pool(name="psS", bufs=2, space="PSUM"))
        psA = ctx.enter_context(tc.tile_pool(name="psA", bufs=2, space="PSUM"))
        psX = ctx.enter_context(tc.tile_pool(name="psX", bufs=2, space="PSUM"))
        pP = ctx.enter_context(tc.tile_pool(name="pP", bufs=4))
        pP2 = ctx.enter_context(tc.tile_pool(name="pP2", bufs=7))
        pA = ctx.enter_context(tc.tile_pool(name="pA", bufs=2))
        pR = ctx.enter_context(tc.tile_pool(name="pR", bufs=4))
        pRB = ctx.enter_context(tc.tile_pool(name="pRB", bufs=2))

        # dummy matmuls warm the PE clock while input DMAs land: the
        # first ~5 run cold (~630ns), then HAM unthrottles and the rest
        # run at ~220ns — ~72 bridges the ~18us xT DMA window
        pw = psX.tile([128, CH], f32, tag="px", name="pw")
        for _ in range(72):
            nc.tensor.matmul(pw[:], junk[:, 0:128], junk[:],
                             start=True, stop=True)

        def qk_proj_m(which, n, m):
            w_t, out_t = (wq, qT) if which == "q" else (wk, kT)
            pp = psX.tile([128, CH], f32, tag="px", name="pp")
            for k in range(KT):
                nc.tensor.matmul(
                    pp[:], w_t[:, k, m * 128:(m + 1) * 128],
                    xT[:, k, n * CH:(n + 1) * CH],
                    start=(k == 0), stop=(k == KT - 1))
            if has_qk_bias:
                b_t = bq if which == "q" else bk
                nc.vector.tensor_scalar_add(
                    out_t[:, m, n * CH:(n + 1) * CH], pp[:],
                    b_t[:, m:m + 1])
            else:
                nc.vector.tensor_copy(
                    out_t[:, m, n * CH:(n + 1) * CH], pp[:])

        def qk_proj_chunk(which, n):
            qk_proj_m(which, n, 0)
            qk_proj_m(which, n, 1)

        def v_proj_tile(s):
            pv = psX.tile([128, HPC, 64], f32, tag="px", name="pv")
            for k in range(KT):
                nc.tensor.matmul(
                    pv[:], xT[:, k, s * 128:(s + 1) * 128], wv[:, k, :],
                    start=(k == 0), stop=(k == KT - 1))
            nc.vector.tensor_copy(va[:, s, :, 0:64], pv[:])

        # upfront: the minimum for attention slot 0 — k-chunk 0 + q0
        # (32 matmuls); k-chunk 1 is only needed by scores group 2 and
        # drains as the first filler, shortening the serial critical
        # path from xT-arrival to the first exp
        qk_proj_chunk("k", 0)
        qk_proj_chunk("q", 0)

        # deferred projection work, drained as PE filler inside the
        # attention loop; ordered by need (k-chunks by scores group,
        # then q1, then v tiles)
        fillers = [lambda m=m: qk_proj_m("k", 1, m) for m in range(2)]
        fillers += [lambda m=m: qk_proj_m("k", 2, m) for m in range(2)]
        fillers += [lambda m=m: qk_proj_m("k", 3, m) for m in range(2)]
        fillers += [lambda m=m: qk_proj_m("q", 1, m) for m in range(2)]
        fillers.extend([lambda s=s: v_proj_tile(s) for s in range(ST)])
        fidx = [0]

        def drain_filler(k=1):
            for _ in range(k):
                if fidx[0] < len(fillers):
                    fillers[fidx[0]]()
                    fidx[0] += 1

        def oproj_piece(nq, att, piece, evac_scalar=False):
            half = nq % 2
            s4, dd = divmod(piece, 2)
            po = psX.tile([128, CH], f32, tag="px", name="po")
            for pr in range(2):
                nc.tensor.matmul(
                    po[:], att[:, pr, s4 * 128:(s4 + 1) * 128],
                    wo[:, pr, dd * CH:(dd + 1) * CH],
                    start=(pr == 0), stop=(pr == 1))
            if evac_scalar:
                nc.scalar.activation(
                    outS[:, half, s4, dd * CH:(dd + 1) * CH], po[:], AF.Copy)
            else:
                nc.vector.tensor_copy(
                    outS[:, half, s4, dd * CH:(dd + 1) * CH], po[:])

        def oproj_dma(nq, lo=0, hi=4):
            half = nq % 2
            nc.sync.dma_start(
                out_e[nq * CH + lo * 128:nq * CH + hi * 128, :].rearrange(
                    "(s p) c -> p s c", p=128),
                outS[:, half, lo:hi, :])

        # flat pipeline over all (chunk, head-pair, sk-group) slots:
        # scores(i), exp/mul(i-1), attn@v(i-3)
        sweeps = [(nq, hp) for nq in range(NQ) for hp in range(2)]
        NS = len(sweeps)
        NSLOT = NS * NG
        sco = {}
        p2s = {}
        accs_by_si = {}
        att_by_nq = {}
        pending_oproj = []
        for slot in range(NSLOT + 3):
            if slot < NSLOT:
                si, g = divmod(slot, NG)
                nq, hp = sweeps[si]
                if g == 0:
                    if hp == 0:
                        if nq == 1:
                            fillers.extend(
                                [lambda m=m: qk_proj_m("q", 2, m)
                                 for m in range(2)])
                        elif nq == 2:
                            fillers.extend(
                                [lambda m=m: qk_proj_m("q", 3, m)
                                 for m in range(2)])
                        att_by_nq[nq] = pA.tile([128, 2, CH], bf16,
                                                name="att", tag="att")
                    accs_by_si[si] = [psA.tile([65, CH], f32, tag="acc",
                                               name=f"acc{j}")
                                      for j in range(2)]
                if slot < 3:
                    # k1/k2/k3 halves: chunk c needed by scores group 2c
                    # (emitted at slot 2c), so 2/slot keeps each ahead
                    drain_filler(2)
                elif slot == 3:
                    # q1 halves + v0,v1: attn@v for slot s (emitted at
                    # iteration s+3) reads v tiles up to 2s+1, so v0/v1
                    # must be emitted here, before attn@v(slot 0)
                    drain_filler(4)
                else:
                    drain_filler(3 if slot < 10 else 2)
                sz = GS[g + 1] - GS[g]
                new = [psS.tile([128, T2, CH], f32, tag="sc", name="sc")
                       for j in range(2)]
                for u in range(sz):
                    t = GS[g] + u
                    for j in range(2):
                        nc.tensor.matmul(
                            new[j][:, u, :],
                            kT[j * 64:(j + 1) * 64, hp,
                               t * 128:(t + 1) * 128],
                            qT[j * 64:(j + 1) * 64, hp,
                               nq * CH:(nq + 1) * CH],
                            start=True, stop=True)
                sco[slot] = new
            if 0 <= slot - 1 < NSLOT:
                sl = slot - 1
                si, g = divmod(sl, NG)
                nq, hp = sweeps[si]
                sz = GS[g + 1] - GS[g]
                cur = []
                for j in range(2):
                    p = pP.tile([128, T2, CH], bf16)
                    nc.scalar.activation(
                        p[:, :sz, :], sco[sl][j][:, :sz, :],
                        AF.Exp, scale=SCALE)
                    p2 = pP2.tile([128, T2, CH], bf16)
                    nc.vector.tensor_mul(
                        p2[:, :sz, :], p[:, :sz, :],
                        eb[:, nq, GS[g]:GS[g] + sz, :])
                    cur.append(p2)
                p2s[sl] = cur
                del sco[sl]
            if 0 <= slot - 3 < NSLOT:
                sl = slot - 3
                si, g = divmod(sl, NG)
                nq, hp = sweeps[si]
                accs = accs_by_si[si]
                sz = GS[g + 1] - GS[g]
                for j in range(2):
                    h = hp * 2 + j
                    for u in range(sz):
                        t = GS[g] + u
                        nc.tensor.matmul(
                            accs[j][:], va[:, t, h, :],
                            p2s[sl][j][:, u, :],
                            start=(t == 0), stop=(t == ST - 1))
                del p2s[sl]
                if pending_oproj:
                    # one o-projection piece per slot: PE filler spread
                    # through the following sweep, deferred so att is
                    # normalized before these reach the queue head.
                    # (Must finish within one sweep: the att tile's pool
                    # buffer is re-allocated two sweeps later.)
                    onq, oatt, pidx, ph = pending_oproj[0]
                    oproj_piece(onq, oatt, pidx,
                                evac_scalar=(pidx % 2 == 0))
                    if pidx == 7:
                        oproj_dma(onq)
                        pending_oproj.pop(0)
                        del att_by_nq[onq]
                    else:
                        pending_oproj[0] = (onq, oatt, pidx + 1, ph + 1)
                if g == NG - 1:
                    # sweep complete: normalize this head-pair's rows.
                    # In the final sweep VectorE is the drain bottleneck,
                    # so the rowsum copies go to the then-idle ScalarE.
                    att = att_by_nq[nq]
                    for j in range(2):
                        rs = pR.tile([1, CH], f32, tag="rs", name="rs")
                        nc.vector.tensor_copy(rs[:], accs[j][64:65, :])
                        r = pR.tile([1, CH], f32)
                        nc.vector.reciprocal_approx_fast(r[:], rs[:])
                        rb = pRB.tile([64, CH], f32)
                        nc.gpsimd.partition_broadcast(rb[:], r[:])
                        nc.vector.tensor_mul(
                            att[j * 64:(j + 1) * 64, hp, :],
                            accs[j][0:64, :], rb[:, :])
                    del accs_by_si[si]
                    if hp == 1:
                        pending_oproj.append((nq, att_by_nq[nq], 0, 0))
        # drain any remaining o-projection (the last chunk): VectorE is
        # the drain bottleneck, so all evacuations go to ScalarE (no
        # exps remain), and the output DMA fires in per-s4 pieces on
        # the idle gpsimd ring to overlap with evacs
        for onq, oatt, pidx, _ph in pending_oproj:
            half = onq % 2
            for piece in range(pidx, 8):
                oproj_piece(onq, oatt, piece, evac_scalar=True)
                if piece % 2 == 1:
                    s4 = piece // 2
                    nc.gpsimd.dma_start(
                        out_e[onq * CH + s4 * 128:
                              onq * CH + (s4 + 1) * 128, :],
                        outS[:, half, s4, :])
    nc.compile()
    return nc


def _get_graph(has_qk_bias):
    key = ("nc", has_qk_bias)
    if key not in _graph_cache:
        _graph_cache[key] = _build(has_qk_bias)
    return _graph_cache[key]


def _prepare_in_maps(inputs, has_qk_bias):
    x = np.asarray(inputs["x"], np.float32)
    bm = np.asarray(inputs["binding_matrix"], np.float32)
    Wq = np.asarray(inputs["Wq"], np.float32)
    Wk = np.asarray(inputs["Wk"], np.float32)
    Wv = np.asarray(inputs["Wv"], np.float32)
    Wo = np.asarray(inputs["Wo"], np.float32)
    bq = np.asarray(inputs["bq"], np.float32)
    bk = np.asarray(inputs["bk"], np.float32)

    expbT = np.exp(0.5 * bm.T).astype(BF16)
    xTs = [np.ascontiguousarray(x[b].T).astype(BF16) for b in range(B)]
    in_maps = []
    for c in range(NCORES):
        b, g = divmod(c, 4)
        sl = slice(g * DHC, (g + 1) * DHC)
        m = {
            "xT": xTs[b],
            "wq": np.ascontiguousarray(Wq[:, sl]).astype(BF16),
            "wk": np.ascontiguousarray(Wk[:, sl]).astype(BF16),
            "wv": np.ascontiguousarray(Wv[:, sl]).astype(BF16),
            "wo": np.ascontiguousarray(Wo[sl, :]).astype(BF16),
            "expbT": expbT,
        }
        if has_qk_bias:
            m["bq"] = np.ascontiguousarray(bq[sl]).reshape(DHC, 1)
            m["bk"] = np.ascontiguousarray(bk[sl]).reshape(DHC, 1)
        in_maps.append(m)
    return in_maps


def _install_trace_hooks():
    """The container image's antenv stub lacks axon_hooks; synthesize it so
    run_bass_kernel_spmd(trace=True) can reach the NTFF profiler in
    libaxon_pjrt.so, and neuter the bucket artifact upload."""
    import types

    try:
        from antenv.axon_hooks import get_axon_ntff_profile_hook  # noqa: F401
    except ImportError:
        import antenv

        m = types.ModuleType("antenv.axon_hooks")
        m._hook = None
        m.set_axon_ntff_profile_hook = lambda h: setattr(m, "_hook", h)
        m.get_axon_ntff_profile_hook = lambda: m._hook
        sys.modules["antenv.axon_hooks"] = m
        antenv.axon_hooks = m
        if "/root/.axon_site" not in sys.path:
            sys.path.insert(0, "/root/.axon_site")
        from trn_agent_boot.trn_boot import _ntff_profile_via_ctypes

        m._hook = _ntff_profile_via_ctypes("/opt/axon/libaxon_pjrt.so")
    import concourse.bass_utils as bu

    bu.upload_artifacts = lambda tmpdir: str(tmpdir)


def run(inputs, trace=False, tmpdir=None):
    from concourse.bass_utils import run_bass_kernel_spmd

    if trace:
        _install_trace_hooks()
    bq = np.asarray(inputs["bq"], np.float32)
    bk = np.asarray(inputs["bk"], np.float32)
    has_qk_bias = bool(np.any(bq) or np.any(bk))
    nc = _get_graph(has_qk_bias)
    in_maps = _prepare_in_maps(inputs, has_qk_bias)
    res = run_bass_kernel_spmd(nc, in_maps, list(range(NCORES)), trace=trace,
                               tmpdir=tmpdir)

    bv = np.asarray(inputs["bv"], np.float32)
    bo = np.asarray(inputs["bo"], np.float32)
    Wo = np.asarray(inputs["Wo"], np.float32)
    const_vec = (bv @ Wo + bo).astype(np.float32)

    out = np.empty((B, S, D), np.float32)
    for b in range(B):
        acc = np.zeros((S, D), np.float32)
        for g in range(4):
            acc += np.asarray(res.results[b * 4 + g]["out"], np.float32)
        out[b] = acc + const_vec
    return out, res


def kernel(**inputs):
    out, _ = run(inputs, trace=False)
    return out


# revision 48
# speedup vs baseline: 1.1823x; 1.1823x over previous
"""AttentionWithBinding distributed Bass kernel for 8 TRN2 NeuronCores.

Sharding: 8 cores = 2 batches x 4 head-groups (4 heads / 256 dims each).
Per core: q/k/v projections (weight-stationary matmuls from a host
pre-transposed xT), flash-style attention in scoresT [sk, sq] orientation,
softmax exp on ScalarE with the additive binding bias folded in as a
host-precomputed exp(0.5*binding.T) bf16 multiplier on VectorE, row-sums
fused into the attn@v matmul via a ones-column on v, and the per-head
o-projection partials. Host sums the 4 partials per batch and adds the
analytic bias vector bv@Wo + bo.

Structure (vs the naive per-chunk loop):
- One flat software pipeline over all (chunk, head-pair, sk-group)
  slots: scores(i), exp/mul(i-1), attn@v(i-3) with no drain/refill at
  head-pair or chunk boundaries, so neither TensorE nor ScalarE (exp is
  ~36us/chunk) ever starves.
- Input DMAs split between the sync HWDGE ring (consolidated multi-MB
  transfers; ~4us fixed cost per DMA) and the gpsimd SWDGE ring
  (fine-grained 128KB tiles at ~0.7us each), ordered by first use.
  Output staged in SBUF, written as one 1MB DMA per chunk.
- The o-projection of chunk n is deferred and dripped one (row,col)
  piece per pipeline slot through chunk n+1, providing exactly the PE
  filler slack the exp/psum alternation needs (kills HAM re-throttle
  micro-gaps); its evacuations alternate ScalarE/VectorE.
- 72 warmup matmuls bridge the initial DMA window to keep the PE clock
  gate warm (the first ~5 run cold, then HAM unthrottles).
"""

import sys

sys.path.insert(0, "/opt/trn_rl_repo")

import numpy as np
import ml_dtypes
from contextlib import ExitStack

BF16 = ml_dtypes.bfloat16

B, S, D = 2, 2048, 1024
H, HD = 16, 64
HPC = 4  # heads per core
DHC = HPC * HD  # 256 head dims per core
SCALE = HD ** -0.5
NCORES = 8
KT = D // 128  # 8 contraction tiles over D
ST = S // 128  # 16 tiles over S
CH = 512  # free-dim chunk (one PSUM bank of f32)
NQ = S // CH  # 4 query chunks

_graph_cache = {}


def _build(has_qk_bias):
    import concourse.bacc as bacc
    import concourse.mybir as mybir
    from concourse import tile

    f32 = mybir.dt.float32
    bf16 = mybir.dt.bfloat16
    AF = mybir.ActivationFunctionType

    nc = bacc.Bacc(None)

    xT_e = nc.declare_dram_parameter("xT", [D, S], bf16, isOutput=False)
    wq_e = nc.declare_dram_parameter("wq", [D, DHC], bf16, isOutput=False)
    wk_e = nc.declare_dram_parameter("wk", [D, DHC], bf16, isOutput=False)
    wv_e = nc.declare_dram_parameter("wv", [D, DHC], bf16, isOutput=False)
    wo_e = nc.declare_dram_parameter("wo", [DHC, D], bf16, isOutput=False)
    eb_e = nc.declare_dram_parameter("expbT", [S, S], bf16, isOutput=False)
    if has_qk_bias:
        bq_e = nc.declare_dram_parameter("bq", [DHC, 1], f32, isOutput=False)
        bk_e = nc.declare_dram_parameter("bk", [DHC, 1], f32, isOutput=False)
    out_e = nc.declare_dram_parameter("out", [S, D], bf16, isOutput=True)

    with tile.TileContext(nc) as tc, ExitStack() as ctx:
        const = ctx.enter_context(tc.tile_pool(name="const", bufs=1))
        xT = const.tile([128, KT, S], bf16)
        eb = const.tile([128, NQ, ST, CH], bf16)  # exp(0.5*binding).T
        wq = const.tile([128, KT, DHC], bf16)
        wk = const.tile([128, KT, DHC], bf16)
        wv = const.tile([128, KT, DHC], bf16)
        junk = const.tile([128, CH], bf16)
        wo = const.tile([128, 2, D], bf16)
        if has_qk_bias:
            bq = const.tile([128, 2], f32)
            bk = const.tile([128, 2], f32)
        qT = const.tile([128, 2, S], bf16)  # [dh, s] head-major
        kT = const.tile([128, 2, S], bf16)
        va = const.tile([128, ST, HPC, 65], bf16)  # v tiles + ones col
        outS = const.tile([128, 2, 4, D], bf16)  # output staging, 2 halves

        nc.vector.memset(junk[:], 0.0)
        # ones columns for the rowsum trick: on the (otherwise idle at
        # start) vector queue so the gpsimd ring is free for DMAs
        for s in range(ST):
            for h in range(HPC):
                nc.vector.memset(va[:, s, h, 64:65], 1.0)

        # DMA plan: sync ring carries the k-projection critical path +
        # late-need tensors; gpsimd ring carries the second xT half and
        # the chunk-0 binding tiles in parallel.
        if has_qk_bias:
            for m in range(2):
                nc.sync.dma_start(bq[:, m:m + 1], bq_e[m * 128:(m + 1) * 128, :])
                nc.sync.dma_start(bk[:, m:m + 1], bk_e[m * 128:(m + 1) * 128, :])
        nc.sync.dma_start(
            wk[:], wk_e[:, :].rearrange("(k p) c -> p k c", p=128))
        nc.sync.dma_start(
            xT[:, 0:4, :],
            xT_e[0:512, :].rearrange("(k p) c -> p k c", p=128))
        nc.gpsimd.dma_start(
            xT[:, 4:8, :],
            xT_e[512:1024, :].rearrange("(k p) c -> p k c", p=128))
        nc.sync.dma_start(
            wq[:], wq_e[:, :].rearrange("(k p) c -> p k c", p=128))
        nc.sync.dma_start(
            wv[:], wv_e[:, :].rearrange("(k p) c -> p k c", p=128))
        for t in range(ST):
            nc.gpsimd.dma_start(
                eb[:, 0, t, :], eb_e[t * 128:(t + 1) * 128, 0:CH])
        nc.sync.dma_start(
            wo[:], wo_e[:, :].rearrange("(m p) c -> p m c", p=128))
        for t in range(ST):
            nc.gpsimd.dma_start(
                eb[:, 1, t, :], eb_e[t * 128:(t + 1) * 128, CH:2 * CH])
        for n in range(2, NQ):
            nc.sync.dma_start(
                eb[:, n, :, :],
                eb_e[:, n * CH:(n + 1) * CH].rearrange(
                    "(t p) c -> p t c", p=128))

        T2 = 2  # sk tiles merged per exp/mul instruction
        GS = list(range(0, ST, T2)) + [ST]
        NG = len(GS) - 1
        psS = ctx.enter_context(tc.tile_pool(name="psS", bufs=2, space="PSUM"))
        psA = ctx.enter_context(tc.tile_pool(name="psA", bufs=2, space="PSUM"))
        psX = ctx.enter_context(tc.tile_pool(name="psX", bufs=2, space="PSUM"))
        pP = ctx.enter_context(tc.tile_pool(name="pP", bufs=4))
        pP2 = ctx.enter_context(tc.tile_pool(name="pP2", bufs=7))
        pA = ctx.enter_context(tc.tile_pool(name="pA", bufs=2))
        pR = ctx.enter_context(tc.tile_pool(name="pR", bufs=4))
        pRB = ctx.enter_context(tc.tile_pool(name="pRB", bufs=2))

        # dummy matmuls warm the PE clock while input DMAs land: the
        # first ~5 run cold (~630ns), then HAM unthrottles and the rest
        # run at ~220ns — ~72 bridges the ~18us xT DMA window
        pw = psX.tile([128, CH], f32, tag="px", name="pw")
        for _ in range(72):
            nc.tensor.matmul(pw[:], junk[:, 0:128], junk[:],
                             start=True, stop=True)

        def qk_proj_m(which, n, m):
            w_t, out_t = (wq, qT) if which == "q" else (wk, kT)
            pp = psX.tile([128, CH], f32, tag="px", name="pp")
            for k in range(KT):
                nc.tensor.matmul(
                    pp[:], w_t[:, k, m * 128:(m + 1) * 128],
                    xT[:, k, n * CH:(n + 1) * CH],
                    start=(k == 0), stop=(k == KT - 1))
            if has_qk_bias:
                b_t = bq if which == "q" else bk
                nc.vector.tensor_scalar_add(
                    out_t[:, m, n * CH:(n + 1) * CH], pp[:],
                    b_t[:, m:m + 1])
            else:
                nc.vector.tensor_copy(
                    out_t[:, m, n * CH:(n + 1) * CH], pp[:])

        def qk_proj_chunk(which, n):
            qk_proj_m(which, n, 0)
            qk_proj_m(which, n, 1)

        def v_proj_tile(s):
            pv = psX.tile([128, HPC, 64], f32, tag="px", name="pv")
            for k in range(KT):
                nc.tensor.matmul(
                    pv[:], xT[:, k, s * 128:(s + 1) * 128], wv[:, k, :],
                    start=(k == 0), stop=(k == KT - 1))
            nc.vector.tensor_copy(va[:, s, :, 0:64], pv[:])

        # upfront: the minimum for attention slot 0 — k-chunk 0 + q0
        # (32 matmuls of serial critical path after xT lands)
        qk_proj_chunk("k", 0)
        qk_proj_chunk("q", 0)

        # deferred projection work, drained as PE filler inside the
        # attention loop. k-chunks are INTERLEAVED with v tiles so no
        # multi-chunk lump sits between early score slots in the PE
        # queue (a k1+k2+k3 lump starves the exp pipeline)
        fillers = [lambda m=m: qk_proj_m("k", 1, m) for m in range(2)]
        fillers += [lambda s=s: v_proj_tile(s) for s in range(2)]
        fillers += [lambda m=m: qk_proj_m("k", 2, m) for m in range(2)]
        fillers += [lambda s=s: v_proj_tile(s) for s in range(2, 4)]
        fillers += [lambda m=m: qk_proj_m("k", 3, m) for m in range(2)]
        fillers += [lambda m=m: qk_proj_m("q", 1, m) for m in range(2)]
        fillers.extend([lambda s=s: v_proj_tile(s) for s in range(4, ST)])
        fidx = [0]

        def drain_filler(k=1):
            for _ in range(k):
                if fidx[0] < len(fillers):
                    fillers[fidx[0]]()
                    fidx[0] += 1

        def oproj_piece(nq, att, piece, evac_scalar=False):
            half = nq % 2
            s4, dd = divmod(piece, 2)
            po = psX.tile([128, CH], f32, tag="px", name="po")
            for pr in range(2):
                nc.tensor.matmul(
                    po[:], att[:, pr, s4 * 128:(s4 + 1) * 128],
                    wo[:, pr, dd * CH:(dd + 1) * CH],
                    start=(pr == 0), stop=(pr == 1))
            if evac_scalar:
                nc.scalar.activation(
                    outS[:, half, s4, dd * CH:(dd + 1) * CH], po[:], AF.Copy)
            else:
                nc.vector.tensor_copy(
                    outS[:, half, s4, dd * CH:(dd + 1) * CH], po[:])

        def oproj_dma(nq, lo=0, hi=4):
            half = nq % 2
            nc.sync.dma_start(
                out_e[nq * CH + lo * 128:nq * CH + hi * 128, :].rearrange(
                    "(s p) c -> p s c", p=128),
                outS[:, half, lo:hi, :])

        # flat pipeline over all (chunk, head-pair, sk-group) slots:
        # scores(i), exp/mul(i-1), attn@v(i-3)
        sweeps = [(nq, hp) for nq in range(NQ) for hp in range(2)]
        NS = len(sweeps)
        NSLOT = NS * NG
        sco = {}
        p2s = {}
        accs_by_si = {}
        att_by_nq = {}
        pending_oproj = []
        for slot in range(NSLOT + 3):
            if slot < NSLOT:
                si, g = divmod(slot, NG)
                nq, hp = sweeps[si]
                if g == 0:
                    if hp == 0:
                        if nq == 1:
                            fillers.extend(
                                [lambda m=m: qk_proj_m("q", 2, m)
                                 for m in range(2)])
                        elif nq == 2:
                            fillers.extend(
                                [lambda m=m: qk_proj_m("q", 3, m)
                                 for m in range(2)])
                        att_by_nq[nq] = pA.tile([128, 2, CH], bf16,
                                                name="att", tag="att")
                    accs_by_si[si] = [psA.tile([65, CH], f32, tag="acc",
                                               name=f"acc{j}")
                                      for j in range(2)]
                if slot < 3:
                    # k2/k3/q1 halves only until wv lands; 2/slot so the
                    # v projections start at slot 3 (v tiles must be
                    # emitted before the attn@v that reads them: attn@v
                    # for slot s needs v up to tile 2s+1 by iteration s+3)
                    drain_filler(2)
                else:
                    drain_filler(3 if slot < 10 else 2)
                sz = GS[g + 1] - GS[g]
                new = [psS.tile([128, T2, CH], f32, tag="sc", name="sc")
                       for j in range(2)]
                for u in range(sz):
                    t = GS[g] + u
                    for j in range(2):
                        nc.tensor.matmul(
                            new[j][:, u, :],
                            kT[j * 64:(j + 1) * 64, hp,
                               t * 128:(t + 1) * 128],
                            qT[j * 64:(j + 1) * 64, hp,
                               nq * CH:(nq + 1) * CH],
                            start=True, stop=True)
                sco[slot] = new
            if 0 <= slot - 1 < NSLOT:
                sl = slot - 1
                si, g = divmod(sl, NG)
                nq, hp = sweeps[si]
                sz = GS[g + 1] - GS[g]
                cur = []
                for j in range(2):
                    p = pP.tile([128, T2, CH], bf16)
                    nc.scalar.activation(
                        p[:, :sz, :], sco[sl][j][:, :sz, :],
                        AF.Exp, scale=SCALE)
                    p2 = pP2.tile([128, T2, CH], bf16)
                    nc.vector.tensor_mul(
                        p2[:, :sz, :], p[:, :sz, :],
                        eb[:, nq, GS[g]:GS[g] + sz, :])
                    cur.append(p2)
                p2s[sl] = cur
                del sco[sl]
            if 0 <= slot - 3 < NSLOT:
                sl = slot - 3
                si, g = divmod(sl, NG)
                nq, hp = sweeps[si]
                accs = accs_by_si[si]
                sz = GS[g + 1] - GS[g]
                for j in range(2):
                    h = hp * 2 + j
                    for u in range(sz):
                        t = GS[g] + u
                        nc.tensor.matmul(
                            accs[j][:], va[:, t, h, :],
                            p2s[sl][j][:, u, :],
                            start=(t == 0), stop=(t == ST - 1))
                del p2s[sl]
                if pending_oproj:
                    # one o-projection piece per slot: PE filler spread
                    # through the following sweep, deferred so att is
                    # normalized before these reach the queue head.
                    # (Must finish within one sweep: the att tile's pool
                    # buffer is re-allocated two sweeps later.)
                    onq, oatt, pidx, ph = pending_oproj[0]
                    oproj_piece(onq, oatt, pidx,
                                evac_scalar=(pidx % 2 == 0))
                    if pidx == 7:
                        oproj_dma(onq)
                        pending_oproj.pop(0)
                        del att_by_nq[onq]
                    else:
                        pending_oproj[0] = (onq, oatt, pidx + 1, ph + 1)
                if g == NG - 1:
                    # sweep complete: normalize this head-pair's rows.
                    # In the final sweep VectorE is the drain bottleneck,
                    # so the rowsum copies go to the then-idle ScalarE.
                    att = att_by_nq[nq]
                    for j in range(2):
                        rs = pR.tile([1, CH], f32, tag="rs", name="rs")
                        nc.vector.tensor_copy(rs[:], accs[j][64:65, :])
                        r = pR.tile([1, CH], f32)
                        nc.vector.reciprocal_approx_fast(r[:], rs[:])
                        rb = pRB.tile([64, CH], f32)
                        nc.gpsimd.partition_broadcast(rb[:], r[:])
                        nc.vector.tensor_mul(
                            att[j * 64:(j + 1) * 64, hp, :],
                            accs[j][0:64, :], rb[:, :])
                    del accs_by_si[si]
                    if hp == 1:
                        pending_oproj.append((nq, att_by_nq[nq], 0, 0))
        # drain any remaining o-projection (the last chunk): VectorE is
        # the drain bottleneck, so all evacuations go to ScalarE (no
        # exps remain), and the output DMA fires in per-s4 pieces on
        # the idle gpsimd ring to overlap with evacs
        for onq, oatt, pidx, _ph in pending_oproj:
            half = onq % 2
            for piece in range(pidx, 8):
                oproj_piece(onq, oatt, piece, evac_scalar=True)
                if piece % 2 == 1:
                    s4 = piece // 2
                    nc.gpsimd.dma_start(
                        out_e[onq * CH + s4 * 128:
                              onq * CH + (s4 + 1) * 128, :],
                        outS[:, half, s4, :])
    nc.compile()
    return nc


def _get_graph(has_qk_bias):
    key = ("nc", has_qk_bias)
    if key not in _graph_cache:
        _graph_cache[key] = _build(has_qk_bias)
    return _graph_cache[key]


def _prepare_in_maps(inputs, has_qk_bias):
    x = np.asarray(inputs["x"], np.float32)
    bm = np.asarray(inputs["binding_matrix"], np.float32)
    Wq = np.asarray(inputs["Wq"], np.float32)
    Wk = np.asarray(inputs["Wk"], np.float32)
    Wv = np.asarray(inputs["Wv"], np.float32)
    Wo = np.asarray(inputs["Wo"], np.float32)
    bq = np.asarray(inputs["bq"], np.float32)
    bk = np.asarray(inputs["bk"], np.float32)

    expbT = np.exp(0.5 * bm.T).astype(BF16)
    xTs = [np.ascontiguousarray(x[b].T).astype(BF16) for b in range(B)]
    in_maps = []
    for c in range(NCORES):
        b, g = divmod(c, 4)
        sl = slice(g * DHC, (g + 1) * DHC)
        m = {
            "xT": xTs[b],
            "wq": np.ascontiguousarray(Wq[:, sl]).astype(BF16),
            "wk": np.ascontiguousarray(Wk[:, sl]).astype(BF16),
            "wv": np.ascontiguousarray(Wv[:, sl]).astype(BF16),
            "wo": np.ascontiguousarray(Wo[sl, :]).astype(BF16),
            "expbT": expbT,
        }
        if has_qk_bias:
            m["bq"] = np.ascontiguousarray(bq[sl]).reshape(DHC, 1)
            m["bk"] = np.ascontiguousarray(bk[sl]).reshape(DHC, 1)
        in_maps.append(m)
    return in_maps


def _install_trace_hooks():
    """The container image's antenv stub lacks axon_hooks; synthesize it so
    run_bass_kernel_spmd(trace=True) can reach the NTFF profiler in
    libaxon_pjrt.so, and neuter the bucket artifact upload."""
    import types

    try:
        from antenv.axon_hooks import get_axon_ntff_profile_hook  # noqa: F401
    except ImportError:
        import antenv

        m = types.ModuleType("antenv.axon_hooks")
        m._hook = None
        m.set_axon_ntff_profile_hook = lambda h: setattr(m, "_hook", h)
        m.get_axon_ntff_profile_hook = lambda: m._hook
        sys.modules["antenv.axon_hooks"] = m
        antenv.axon_hooks = m
        if "/root/.axon_site" not in sys.path:
            sys.path.insert(0, "/root/.axon_site")
        from trn_agent_boot.trn_boot import _ntff_profile_via_ctypes

        m._hook = _ntff_profile_via_ctypes("/opt/axon/libaxon_pjrt.so")
    import concourse.bass_utils as bu

    bu.upload_artifacts = lambda tmpdir: str(tmpdir)


def run(inputs, trace=False, tmpdir=None):
    from concourse.bass_utils import run_bass_kernel_spmd

    if trace:
        _install_trace_hooks()
    bq = np.asarray(inputs["bq"], np.float32)
    bk = np.asarray(inputs["bk"], np.float32)
    has_qk_bias = bool(np.any(bq) or np.any(bk))
    nc = _get_graph(has_qk_bias)
    in_maps = _prepare_in_maps(inputs, has_qk_bias)
    res = run_bass_kernel_spmd(nc, in_maps, list(range(NCORES)), trace=trace,
                               tmpdir=tmpdir)

    bv = np.asarray(inputs["bv"], np.float32)
    bo = np.asarray(inputs["bo"], np.float32)
    Wo = np.asarray(inputs["Wo"], np.float32)
    const_vec = (bv @ Wo + bo).astype(np.float32)

    out = np.empty((B, S, D), np.float32)
    for b in range(B):
        acc = np.zeros((S, D), np.float32)
        for g in range(4):
            acc += np.asarray(res.results[b * 4 + g]["out"], np.float32)
        out[b] = acc + const_vec
    return out, res


def kernel(**inputs):
    out, _ = run(inputs, trace=False)
    return out
